# revision 13
# baseline (speedup 1.0000x reference)
"""Trainium2 Bass kernel for a causal self-attention block (GQA + per-head
RMS-norm + RoPE + learned q-gain), sharded over 8 NeuronCores.

Sharding: data-parallel over batch (B=2) x tensor-parallel over KV groups
(4 groups of 4 query heads). core = b*4 + g. Each core computes attention for
its 4 heads and a partial output projection (its 256 in-dims of Wproj); the
host sums the 4 partials per batch element.

This version is a software-pipelined rewrite tuned for engine balance:
  - one fused instruction stream: QKV chunks (U), attention blocks (B) and
    output-projection blocks (C) interleave so the PE never idles long enough
    to drop out of its warm clock state.
  - q is stored in head-PAIR layout (partitions 0-63 = even head dims,
    64-127 = odd head dims, straight out of a single 128x128 PE transpose);
    two zero-padded copies of k^T (kTe: k in rows 0-63, kTo: rows 64-127)
    let each head's score matmul contract only its half.
  - attention works on the causal band only: score/exp/mask/PV widths shrink
    on diagonal tiles, with a single shared triangular mask.
  - RMS-norm rsqrt is computed with a bitcast magic-constant seed + Newton
    steps on the DVE, so the scalar engine only ever loads the exp table set.
  - p/v/y/Wproj run in bf16 (fp32 PSUM accumulation); q/k scores stay fp32r.
  - softmax denominator rides row 0 of the PV matmul via a ones-column in V;
    reciprocal_approx_fast + gpsimd partition_broadcast normalize it.
"""

import math

import numpy as np

import concourse.bacc as bacc
import concourse.bass as bass
import concourse.tile as tile
from concourse import mybir
from concourse.bass import ts
from concourse.bass_utils import run_bass_kernel_spmd
from concourse.masks import make_identity

# Problem dims (hardcoded per contract).
B, S, D, H, KV, HD = 2, 2048, 1024, 16, 4, 64
NH = H // KV          # 4 query heads per core (one KV group)
GD = NH * HD          # 256 out-dims of Wq per group
P = 128               # partitions
NST = S // P          # 16 sequence tiles
JW = 512              # query-block width for attention
NJ = S // JW          # 4 query blocks
NC = 8                # cores
ROPE_BASE = 10000.0
RMS_EPS = 1.1920929e-07
F32 = mybir.dt.float32
F32R = mybir.dt.float32r
BF16 = mybir.dt.bfloat16
I32 = mybir.dt.int32
AXX = mybir.AxisListType.X
ACT = mybir.ActivationFunctionType
ALU = mybir.AluOpType
NQKV = GD + 2 * HD    # 384


def _build_program(reps=1):
    nc = bacc.Bacc("TRN2", target_bir_lowering=False, debug=False)

    xT = nc.dram_tensor("xT", [D, S], BF16, kind="ExternalInput").ap()
    wqkv = nc.dram_tensor("wqkv", [D, NQKV], BF16, kind="ExternalInput").ap()
    wp2 = nc.dram_tensor("wp2", [P, 2 * D], BF16, kind="ExternalInput").ap()
    cos1 = nc.dram_tensor("cos1", [P, NST * HD], F32, kind="ExternalInput").ap()
    sin1 = nc.dram_tensor("sin1", [P, NST * 32], F32, kind="ExternalInput").ap()
    tri = nc.dram_tensor("tri", [P, JW], BF16, kind="ExternalInput").ap()
    qg8 = nc.dram_tensor("qg8", [1, NH], F32, kind="ExternalInput").ap()
    ypt = nc.dram_tensor("ypt", [D, S], BF16, kind="ExternalOutput").ap()
    dnb = nc.dram_tensor("dnb", [8, 2 * JW], F32, kind="Internal").ap()

    with tile.TileContext(nc) as tc:
        for _ in range(reps):
            _body(tc, xT, wqkv, wp2, cos1, sin1, tri, qg8, ypt, dnb)
    nc.compile()
    return nc


def _body(tc, xT, wqkv, wp2, cos1, sin1, tri, qg8, ypt, dnb):
    nc = tc.nc
    xTr = xT.rearrange("(c p) s -> p c s", p=P)

    with (
        tc.tile_pool(name="consts", bufs=1) as consts,
        tc.tile_pool(name="wk", bufs=3) as wk,
        tc.tile_pool(name="rwk", bufs=4) as rwk,
        tc.tile_pool(name="pwk", bufs=3) as pwk,
        tc.tile_pool(name="nwk", bufs=2) as nwk,
        tc.tile_pool(name="psmisc", bufs=2, space="PSUM") as psmisc,
        tc.tile_pool(name="psst", bufs=2, space="PSUM") as psst,
        tc.tile_pool(name="psy", bufs=1, space="PSUM") as psy,
    ):
        # ---------------- persistent SBUF state ----------------
        xt_all = consts.tile([P, 8, S], BF16, name="xt_all")
        w_sb = consts.tile([P, 8, NQKV], BF16, name="w_sb")
        wp_sb = consts.tile([P, 2, D], BF16, name="wp_sb")
        cos_sb = consts.tile([P, NST, HD], F32, name="cos_sb")
        sin_sb = consts.tile([P, NST, 32], F32, name="sin_sb")
        tri_sb = consts.tile([P, JW], BF16, name="tri_sb")
        qg8_sb = consts.tile([P, NH], F32, name="qg8_sb")
        ident = consts.tile([P, P], BF16, name="ident")
        negI = consts.tile([P, P], BF16, name="negI")
        qT2 = consts.tile([P, 2, S], BF16, name="qT2")
        kTe = consts.tile([P, S], BF16, name="kTe")
        kTo = consts.tile([P, S], BF16, name="kTo")
        # PV stationary operand: col 0 = ones (softmax denominator -> PSUM row
        # 0, where the custom recip/broadcast ops are legal), cols 1-63 = zero,
        # cols 64-127 = v dims (y lands at rows 64-127, 32-aligned).
        v_sb = consts.tile([P, NST, P], BF16, name="v_sb")
        y_sb = consts.tile([P, 2, S], BF16, name="y_sb")
        qkv_sb = consts.tile([P, NST, 5 * HD], F32, name="qkv_sb")
        ss_all = consts.tile([P, NST * 5], F32, name="ss_all")
        r_all = consts.tile([P, NST * 5], F32, name="r_all")

        # First the 128-col x slice + weights the first QKV matmul needs,
        # then the RoPE tables (used at ~7us), then the rest of x.
        wqr = wqkv.rearrange("(c p) n -> p c n", p=P)
        dmaq = [nc.sync, nc.scalar, nc.gpsimd]
        for c in range(8):
            dmaq[c % 2].dma_start(out=xt_all[:, c, 0:P], in_=xTr[:, c, 0:P])
        for c in range(8):
            dmaq[c % 3].dma_start(out=w_sb[:, c, :], in_=wqr[:, c, :])
        nc.gpsimd.dma_start(out=qg8_sb, in_=qg8.to_broadcast([P, NH]))
        nc.gpsimd.dma_start(out=tri_sb, in_=tri)
        nc.scalar.dma_start(
            out=cos_sb, in_=cos1.rearrange("p (t f) -> p t f", t=NST)
        )
        nc.sync.dma_start(
            out=sin_sb, in_=sin1.rearrange("p (t f) -> p t f", t=NST)
        )
        for c in range(8):
            dmaq[c % 3].dma_start(
                out=xt_all[:, c, P : 9 * P], in_=xTr[:, c, P : 9 * P]
            )
        for c in range(8):
            dmaq[(c + 1) % 3].dma_start(
                out=xt_all[:, c, 9 * P : S], in_=xTr[:, c, 9 * P : S]
            )
        make_identity(nc, ident)
        nc.vector.tensor_scalar(
            out=negI, in0=ident, scalar1=-1.0, scalar2=None, op0=ALU.mult
        )

        # zero fills via memset on a bitcast view (f32r cannot be memset
        # directly); ones/zeros for the PV operand are plain bf16 memsets.
        nc.gpsimd.memset(kTe[HD:P, :], 0.0)
        nc.gpsimd.memset(kTo[0:HD, :], 0.0)
        nc.gpsimd.memset(v_sb[:, :, 0:1], 1.0)
        nc.gpsimd.memset(v_sb[:, :, 1:HD], 0.0)
        nc.sync.dma_start(out=wp_sb, in_=wp2.rearrange("p (c m) -> p c m", c=2))

        # ---------------- pipelined stream ----------------
        # The attention (B) blocks are rate-limited by the scalar engine's
        # exps, so every independent PE work item (QKV matmuls, output
        # projection) is wrapped in a closure and sprinkled INTO the B tile
        # loops ("fillers") to keep the tensor engine dense (HAM stays warm).
        rot_tiles = {}

        def u_tile(jb, il):
            def go():
                i = 4 * jb + il
                qkv_ps = psmisc.tile([P, NQKV], F32, name=f"qkv{i}", tag="mi")
                for c in range(8):
                    nc.tensor.matmul(
                        qkv_ps,
                        lhsT=xt_all[:, c, ts(i, P)],
                        rhs=w_sb[:, c, :],
                        start=(c == 0),
                        stop=(c == 7),
                    )
                # stage q,k (f32) and v (bf16); square+reduce for RMS stats
                nc.vector.tensor_copy(qkv_sb[:, i, :], qkv_ps[:, 0 : 5 * HD])
                nc.vector.tensor_copy(v_sb[:, i, HD:P], qkv_ps[:, 5 * HD : NQKV])
                sq = wk.tile([P, 5 * HD], F32, name=f"sq{i}", tag="sq")
                nc.gpsimd.tensor_mul(sq, qkv_sb[:, i, :], qkv_sb[:, i, :])
                nc.vector.reduce_sum(
                    ss_all[:, 5 * i : 5 * i + 5],
                    sq.rearrange("p (h d) -> p h d", d=HD),
                    axis=AXX,
                )
            return go

        def u_post(jb):
            def go():
                # rsqrt via bitcast magic seed + 2 Newton steps (DVE)
                ssc = ss_all[:, 20 * jb : 20 * jb + 20]
                rc = r_all[:, 20 * jb : 20 * jb + 20]
                mm = wk.tile([P, 20], F32, name=f"m{jb}", tag="m")
                nc.vector.tensor_scalar(
                    out=mm, in0=ssc, scalar1=1.0 / HD, scalar2=RMS_EPS,
                    op0=ALU.mult, op1=ALU.add,
                )
                tt = wk.tile([P, 20], F32, name=f"t{jb}", tag="t")
                nc.vector.tensor_scalar(
                    out=tt.bitcast(I32), in0=mm.bitcast(I32),
                    scalar1=1, scalar2=-1,
                    op0=ALU.logical_shift_right, op1=ALU.bitwise_xor,
                )
                nc.vector.tensor_scalar(
                    out=rc.bitcast(I32), in0=tt.bitcast(I32),
                    scalar1=0x5F3759E0, scalar2=None, op0=ALU.add,
                )
                for _ in range(2):
                    nc.vector.tensor_mul(tt, rc, rc)
                    nc.vector.tensor_mul(tt, tt, mm)
                    nc.vector.tensor_scalar(
                        out=tt, in0=tt, scalar1=-0.5, scalar2=1.5,
                        op0=ALU.mult, op1=ALU.add,
                    )
                    nc.vector.tensor_mul(rc, rc, tt)
                rcv = rc.rearrange("p (t h) -> p t h", h=5)
                nc.vector.tensor_mul(
                    rcv[:, :, 0:NH], rcv[:, :, 0:NH],
                    qg8_sb[:, None, :].broadcast_to([P, 4, NH]),
                )
                # RoPE, mostly on the Pool engine (it is otherwise idle; the
                # DVE is loaded with masks/normalization/copies)
                qc = qkv_sb[:, 4 * jb : 4 * jb + 4, :]
                qcv = qc.rearrange("p t (h d) -> p (t h) d", d=HD)
                qks = rwk.tile([P, 4, 5 * HD], F32, name=f"qks{jb}", tag="qks")
                qksv = qks.rearrange("p t (h d) -> p (t h) d", d=HD)
                nc.vector.tensor_mul(
                    qksv, qcv,
                    rcv.rearrange("p t h -> p (t h)")[:, :, None].broadcast_to([P, 20, HD]),
                )
                rot = rwk.tile([P, 4, 5 * HD], BF16, name=f"rot{jb}", tag="rot")
                qks4 = qks.rearrange("p t (h d) -> p t h d", d=HD)
                rot4 = rot.rearrange("p t (h d) -> p t h d", d=HD)
                cosb = cos_sb[:, 4 * jb : 4 * jb + 4, None, :].broadcast_to(
                    [P, 4, 5, HD]
                )
                nc.vector.tensor_mul(rot4, qks4, cosb)
                rotv = rot.rearrange("p t (h d) -> p (t h) d", d=HD)
                sinb = sin_sb[:, 4 * jb : 4 * jb + 4, None, :].broadcast_to(
                    [P, 4, 5, 32]
                )
                m2a = rwk.tile([P, 20, 32], BF16, name=f"m2a{jb}", tag="m2a")
                nc.gpsimd.tensor_mul(
                    m2a.rearrange("p (t h) d -> p t h d", h=5),
                    qks4[:, :, :, 32:HD], sinb,
                )
                m2b = rwk.tile([P, 20, 32], BF16, name=f"m2b{jb}", tag="m2b")
                nc.gpsimd.tensor_mul(
                    m2b.rearrange("p (t h) d -> p t h d", h=5),
                    qks4[:, :, :, 0:32], sinb,
                )
                nc.vector.tensor_add(rotv[:, :, 0:32], rotv[:, :, 0:32], m2a)
                nc.vector.tensor_sub(rotv[:, :, 32:HD], rotv[:, :, 32:HD], m2b)
                rot_tiles[jb] = rot
            return go

        def tr_chunk(jb):
            rot = rot_tiles[jb]
            for il in range(4):
                i = 4 * jb + il
                for pair in range(2):
                    trp = psmisc.tile([P, P], BF16, name=f"tr{i}_{pair}", tag="mi")
                    nc.tensor.transpose(trp, rot[:, il, ts(pair, P)], ident)
                    nc.vector.tensor_copy(qT2[:, pair, ts(i, P)], trp)
                trk = psmisc.tile([HD, P], BF16, name=f"trk{i}", tag="mi")
                nc.tensor.transpose(trk, rot[:, il, 4 * HD : 5 * HD], ident)
                nc.vector.tensor_copy(kTe[0:HD, ts(i, P)], trk)
            nc.gpsimd.dma_start(
                out=kTo[HD:P, 4 * jb * P : (4 * jb + 4) * P],
                in_=kTe[0:HD, 4 * jb * P : (4 * jb + 4) * P],
            )

        def c_tile(j, mtile):
            def go():
                op = psmisc.tile([P, JW], F32, name=f"op{j}_{mtile}", tag="mi")
                for c in range(2):
                    nc.tensor.matmul(
                        op,
                        lhsT=wp_sb[:, c, ts(mtile, P)],
                        rhs=y_sb[:, c, ts(j, JW)],
                        start=(c == 0),
                        stop=(c == 1),
                    )
                o_sb = nwk.tile([P, JW], BF16, name=f"o{j}_{mtile}", tag="o")
                if mtile % 2 == 0:
                    nc.vector.tensor_copy(o_sb, op)
                else:
                    nc.scalar.copy(o_sb, op)
                [nc.sync, nc.scalar, nc.gpsimd][mtile % 3].dma_start(
                    out=ypt[ts(mtile, P), ts(j, JW)], in_=o_sb
                )
            return go

        def b_block(j, fillers):
            """Attention for q-block j; pops one filler after each tile."""
            nt = 4 * (j + 1)
            for pair in range(2):
                yp = psy.tile([P, 2, JW], F32, name=f"y{j}_{pair}", tag="y")
                pend = []

                def pv_flush():
                    pt, pw_, pc0 = pend.pop(0)
                    nc.tensor.matmul(
                        yp[:, 0, pc0:JW], lhsT=v_sb[:, pt, :], rhs=pw_[:, 0, pc0:JW],
                        start=(pt == 0), stop=(pt == nt - 1),
                    )
                    nc.tensor.matmul(
                        yp[:, 1, pc0:JW], lhsT=v_sb[:, pt, :], rhs=pw_[:, 1, pc0:JW],
                        start=(pt == 0), stop=(pt == nt - 1),
                    )

                for t in range(nt):
                    m = t - 4 * j
                    w = JW if m < 0 else JW - P * m
                    c0 = JW - w
                    st = psst.tile([P, 2, JW], F32, name=f"st{j}_{pair}_{t}", tag="st")
                    p_sb = pwk.tile([P, 2, JW], BF16, name=f"p{j}_{pair}_{t}", tag="p")
                    qe = qT2[:, pair, ts(j, JW)]
                    diag = m >= 0
                    nc.tensor.matmul(
                        st[:, 0, c0:JW], lhsT=kTe[:, ts(t, P)], rhs=qe[:, c0:JW],
                        start=True, stop=not diag, skip_group_check=diag,
                    )
                    nc.tensor.matmul(
                        st[:, 1, c0:JW], lhsT=kTo[:, ts(t, P)], rhs=qe[:, c0:JW],
                        start=True, stop=not diag, skip_group_check=diag,
                    )
                    if diag:
                        nc.tensor.matmul(
                            st[:, 0, c0 : c0 + P], lhsT=negI, rhs=tri_sb[:, 0:P],
                            start=False, stop=True, skip_group_check=True,
                        )
                        nc.tensor.matmul(
                            st[:, 1, c0 : c0 + P], lhsT=negI, rhs=tri_sb[:, 0:P],
                            start=False, stop=True, skip_group_check=True,
                        )
                    if fillers:
                        fillers.pop(0)()
                    if len(pend) >= 2:
                        pv_flush()
                    nc.scalar.activation(p_sb[:, :, c0:JW], st[:, :, c0:JW], ACT.Exp)
                    pend.append((t, p_sb, c0))
                while pend:
                    pv_flush()
                # softmax normalization: row 0 of each head-half holds the
                # denominator; y dims sit at rows 64-127 (aligned).
                slot = 2 * j + pair
                rcp = nwk.tile([1, 2, JW], F32, name=f"rc{j}_{pair}", tag="rcp")
                nc.vector.reciprocal_approx_fast(rcp, yp[0:1, :, :])
                nc.sync.dma_start(out=dnb[slot : slot + 1, :], in_=rcp)
                bc = nwk.tile([P, 2, JW], F32, name=f"bc{j}_{pair}", tag="bc")
                nc.gpsimd.dma_start(
                    out=bc[HD:P, :, :],
                    in_=dnb[slot : slot + 1, :].rearrange("o (h w) -> o h w", h=2)
                    .to_broadcast([HD, 2, JW]),
                )
                ytp = nwk.tile([P, 2, JW], BF16, name=f"yt{j}_{pair}", tag="yt")
                nc.vector.tensor_mul(
                    ytp[HD:P, :, :], yp[HD:P, :, :], bc[HD:P, :, :]
                )
                nc.sync.dma_start(
                    out=y_sb[0:HD, pair, ts(j, JW)], in_=ytp[HD:P, 0, :]
                )
                nc.sync.dma_start(
                    out=y_sb[HD:P, pair, ts(j, JW)], in_=ytp[HD:P, 1, :]
                )

        # program order (v7-style software pipeline): transposes decoupled
        # from their U chunks, C blocks pulled ahead of the final B block.
        for il in range(4):
            u_tile(0, il)()
        u_post(0)()
        tr_chunk(0)
        for il in range(4):
            u_tile(1, il)()
        b_block(0, [])
        u_post(1)()
        for il in range(4):
            u_tile(2, il)()
        tr_chunk(1)
        b_block(1, [])
        u_post(2)()
        for il in range(4):
            u_tile(3, il)()
        tr_chunk(2)
        for m in range(8):
            c_tile(0, m)()
        b_block(2, [u_post(3)])
        tr_chunk(3)
        for m in range(8):
            c_tile(1, m)()
        for m in range(8):
            c_tile(2, m)()
        b_block(3, [])
        for m in range(8):
            c_tile(3, m)()


_PROG = None


def _get_program():
    global _PROG
    if _PROG is None:
        _PROG = _build_program()
    return _PROG


def _bf16(a):
    import ml_dtypes

    return np.ascontiguousarray(a.astype(ml_dtypes.bfloat16))


def _host_tables():
    inv_freq = (
        1.0 / (ROPE_BASE ** (np.arange(0, HD, 2, dtype=np.float32) / HD))
    ).astype(np.float32)
    t = np.arange(S, dtype=np.float32)
    freqs = t[:, None] * inv_freq[None, :]  # [S, 32]
    cosf = np.cos(freqs).astype(np.float32)
    sinf = np.sin(freqs).astype(np.float32)
    cosd = np.concatenate([cosf, cosf], axis=1)  # [S, 64]
    cos1 = np.ascontiguousarray(
        cosd.reshape(NST, P, HD).transpose(1, 0, 2).reshape(P, NST * HD)
    )
    sin1 = np.ascontiguousarray(
        sinf.reshape(NST, P, 32).transpose(1, 0, 2).reshape(P, NST * 32)
    )
    p_idx = np.arange(P)[:, None]
    x_idx = np.arange(JW)[None, :]
    # complement causal mask: +30000 where masked; subtracted from scores via
    # a PE accumulate matmul with -I, so exp underflows to exactly 0 there.
    tri = _bf16(np.where(x_idx < p_idx, 30000.0, 0.0))  # [128, 512]
    return cos1, sin1, tri


def _in_maps(x, Wq, Wk, Wv, Wproj, q_gain):
    cos1, sin1, tri = _host_tables()
    maps = []
    for core in range(NC):
        b, g = divmod(core, KV)
        xT = _bf16(x[b].T)  # [D, S]
        wqkv = _bf16(
            np.concatenate(
                [
                    Wq[g * GD : (g + 1) * GD].T,
                    Wk[g * HD : (g + 1) * HD].T,
                    Wv[g * HD : (g + 1) * HD].T,
                ],
                axis=1,
            )
        )  # [D, 384]
        wsl = Wproj[:, g * GD : (g + 1) * GD].T.reshape(NH, HD, D)  # [head, d, m]
        wp2 = _bf16(
            np.stack(
                [
                    np.concatenate([wsl[0], wsl[1]], axis=0),
                    np.concatenate([wsl[2], wsl[3]], axis=0),
                ],
                axis=1,
            ).reshape(P, 2 * D)
        )
        qg8 = np.ascontiguousarray(
            (q_gain[g * NH : (g + 1) * NH] / 8.0).astype(np.float32).reshape(1, NH)
        )
        maps.append(
            {
                "xT": xT,
                "wqkv": wqkv,
                "wp2": wp2,
                "cos1": cos1,
                "sin1": sin1,
                "tri": tri,
                "qg8": qg8,
            }
        )
    return maps


def kernel(x, Wq, Wk, Wv, Wproj, q_gain, _collect=None):
    x = np.asarray(x, dtype=np.float32)
    Wq = np.asarray(Wq, dtype=np.float32)
    Wk = np.asarray(Wk, dtype=np.float32)
    Wv = np.asarray(Wv, dtype=np.float32)
    Wproj = np.asarray(Wproj, dtype=np.float32)
    q_gain = np.asarray(q_gain, dtype=np.float32)

    nc = _get_program()
    maps = _in_maps(x, Wq, Wk, Wv, Wproj, q_gain)
    res = run_bass_kernel_spmd(nc, maps, core_ids=list(range(NC)))
    if _collect is not None:
        _collect.append(res)

    out = np.zeros((B, S, D), dtype=np.float32)
    for core in range(NC):
        b, _ = divmod(core, KV)
        out[b] += res.results[core]["ypt"].astype(np.float32).T
    return out.astype(np.float32)



# revision 15
# speedup vs baseline: 1.2049x; 1.2049x over previous
"""Trainium2 Bass kernel for a causal self-attention block (GQA + per-head
RMS-norm + RoPE + learned q-gain), sharded over 8 NeuronCores.

Sharding: data-parallel over batch (B=2) x tensor-parallel over KV groups
(4 groups of 4 query heads). core = b*4 + g. Each core computes attention for
its 4 heads and a partial output projection (its 256 in-dims of Wproj); the
host sums the 4 partials per batch element.

This version is a software-pipelined rewrite tuned for engine balance:
  - one fused instruction stream: QKV chunks (U), attention blocks (B) and
    output-projection blocks (C) interleave so the PE never idles long enough
    to drop out of its warm clock state.
  - q is stored in head-PAIR layout (partitions 0-63 = even head dims,
    64-127 = odd head dims, straight out of a single 128x128 PE transpose);
    two zero-padded copies of k^T (kTe: k in rows 0-63, kTo: rows 64-127)
    let each head's score matmul contract only its half.
  - attention works on the causal band only: score/exp/mask/PV widths shrink
    on diagonal tiles, with a single shared triangular mask.
  - RMS-norm rsqrt is computed with a bitcast magic-constant seed + Newton
    steps on the DVE, so the scalar engine only ever loads the exp table set.
  - p/v/y/Wproj run in bf16 (fp32 PSUM accumulation); q/k scores stay fp32r.
  - softmax denominator rides row 0 of the PV matmul via a ones-column in V;
    reciprocal_approx_fast + gpsimd partition_broadcast normalize it.
"""

import math

import numpy as np

import concourse.bacc as bacc
import concourse.bass as bass
import concourse.tile as tile
from concourse import mybir
from concourse.bass import ts
from concourse.bass_utils import run_bass_kernel_spmd
from concourse.masks import make_identity

# Problem dims (hardcoded per contract).
B, S, D, H, KV, HD = 2, 2048, 1024, 16, 4, 64
NH = H // KV          # 4 query heads per core (one KV group)
GD = NH * HD          # 256 out-dims of Wq per group
P = 128               # partitions
NST = S // P          # 16 sequence tiles
JW = 512              # query-block width for attention
NJ = S // JW          # 4 query blocks
NC = 8                # cores
ROPE_BASE = 10000.0
RMS_EPS = 1.1920929e-07
F32 = mybir.dt.float32
F32R = mybir.dt.float32r
BF16 = mybir.dt.bfloat16
I32 = mybir.dt.int32
AXX = mybir.AxisListType.X
ACT = mybir.ActivationFunctionType
ALU = mybir.AluOpType
NQKV = GD + 2 * HD    # 384


def _build_program(reps=1):
    nc = bacc.Bacc("TRN2", target_bir_lowering=False, debug=False)

    xT = nc.dram_tensor("xT", [D, S], BF16, kind="ExternalInput").ap()
    wqkv = nc.dram_tensor("wqkv", [D, NQKV], BF16, kind="ExternalInput").ap()
    wp2 = nc.dram_tensor("wp2", [P, 2 * D], BF16, kind="ExternalInput").ap()
    cos1 = nc.dram_tensor("cos1", [P, NST * HD], F32, kind="ExternalInput").ap()
    sin1 = nc.dram_tensor("sin1", [P, NST * 32], F32, kind="ExternalInput").ap()
    tri = nc.dram_tensor("tri", [P, JW], BF16, kind="ExternalInput").ap()
    qg8 = nc.dram_tensor("qg8", [1, NH], F32, kind="ExternalInput").ap()
    ypt = nc.dram_tensor("ypt", [D, S], BF16, kind="ExternalOutput").ap()
    dnb = nc.dram_tensor("dnb", [8, 2 * JW], F32, kind="Internal").ap()

    with tile.TileContext(nc) as tc:
        for _ in range(reps):
            _body(tc, xT, wqkv, wp2, cos1, sin1, tri, qg8, ypt, dnb)
    nc.compile()
    return nc


def _body(tc, xT, wqkv, wp2, cos1, sin1, tri, qg8, ypt, dnb):
    nc = tc.nc
    xTr = xT.rearrange("(c p) s -> p c s", p=P)

    with (
        tc.tile_pool(name="consts", bufs=1) as consts,
        tc.tile_pool(name="xtp", bufs=3) as xtp,
        tc.tile_pool(name="wk", bufs=3) as wk,
        tc.tile_pool(name="rwk", bufs=4) as rwk,
        tc.tile_pool(name="pwk", bufs=3) as pwk,
        tc.tile_pool(name="nwk", bufs=2) as nwk,
        tc.tile_pool(name="psmisc", bufs=2, space="PSUM") as psmisc,
        tc.tile_pool(name="psst", bufs=2, space="PSUM") as psst,
        tc.tile_pool(name="psy", bufs=1, space="PSUM") as psy,
    ):
        # ---------------- persistent SBUF state ----------------
        w_sb = consts.tile([P, 8, NQKV], BF16, name="w_sb")
        wp_sb = consts.tile([P, 2, D], BF16, name="wp_sb")
        cos_sb = consts.tile([P, NST, HD], F32, name="cos_sb")
        sin_sb = consts.tile([P, NST, 32], F32, name="sin_sb")
        tri_sb = consts.tile([P, JW], BF16, name="tri_sb")
        qg8_sb = consts.tile([P, NH], F32, name="qg8_sb")
        ident = consts.tile([P, P], BF16, name="ident")
        negI = consts.tile([P, P], BF16, name="negI")
        qT2 = consts.tile([P, 2, S], BF16, name="qT2")
        kTe = consts.tile([P, S], BF16, name="kTe")
        kTo = consts.tile([P, S], BF16, name="kTo")
        # PV stationary operand: col 0 = ones (softmax denominator -> PSUM row
        # 0, where the custom recip/broadcast ops are legal), cols 1-63 = zero,
        # cols 64-127 = v dims (y lands at rows 64-127, 32-aligned).
        v_sb = consts.tile([P, NST, P], BF16, name="v_sb")
        y_sb = consts.tile([P, 2, S], BF16, name="y_sb")
        qkv_sb = consts.tile([P, NST, 5 * HD], F32, name="qkv_sb")
        ss_all = consts.tile([P, NST * 5], F32, name="ss_all")
        r_all = consts.tile([P, NST * 5], F32, name="r_all")

        # x-tile and weight DMAs first so the first QKV matmul starts ASAP;
        # wp (needed only by C0) goes last.
        wqr = wqkv.rearrange("(c p) n -> p c n", p=P)
        dmaq = [nc.sync, nc.scalar, nc.gpsimd]
        xt0 = xtp.tile([P, 8, JW], BF16, name="xt0", tag="xt")
        for c in range(8):
            qa, qb = (nc.sync, nc.scalar) if c % 2 == 0 else (nc.scalar, nc.sync)
            qa.dma_start(out=xt0[:, c, :], in_=xTr[:, c, ts(0, JW)])
            qb.dma_start(out=w_sb[:, c, :], in_=wqr[:, c, :])
        nc.gpsimd.dma_start(out=qg8_sb, in_=qg8.to_broadcast([P, NH]))
        nc.gpsimd.dma_start(out=tri_sb, in_=tri)
        nc.scalar.dma_start(
            out=cos_sb, in_=cos1.rearrange("p (t f) -> p t f", t=NST)
        )
        nc.sync.dma_start(
            out=sin_sb, in_=sin1.rearrange("p (t f) -> p t f", t=NST)
        )
        make_identity(nc, ident)
        nc.vector.tensor_scalar(
            out=negI, in0=ident, scalar1=-1.0, scalar2=None, op0=ALU.mult
        )

        # zero fills via memset on a bitcast view (f32r cannot be memset
        # directly); ones/zeros for the PV operand are plain bf16 memsets.
        nc.gpsimd.memset(kTe[HD:P, :], 0.0)
        nc.gpsimd.memset(kTo[0:HD, :], 0.0)
        nc.gpsimd.memset(v_sb[:, :, 0:1], 1.0)
        nc.gpsimd.memset(v_sb[:, :, 1:HD], 0.0)
        nc.sync.dma_start(out=wp_sb, in_=wp2.rearrange("p (c m) -> p c m", c=2))

        # ---------------- pipelined stream ----------------
        # The attention (B) blocks are rate-limited by the scalar engine's
        # exps, so every independent PE work item (QKV matmuls, output
        # projection) is wrapped in a closure and sprinkled INTO the B tile
        # loops ("fillers") to keep the tensor engine dense (HAM stays warm).
        rot_tiles = {}
        xts = {0: xt0}

        def u_tile(jb, il):
            def go():
                if il == 0 and jb + 1 < 4:
                    nxt = xtp.tile([P, 8, JW], BF16, name=f"xt{jb+1}", tag="xt")
                    for c in range(8):
                        q = nc.sync if c % 2 == 0 else nc.scalar
                        q.dma_start(out=nxt[:, c, :], in_=xTr[:, c, ts(jb + 1, JW)])
                    xts[jb + 1] = nxt
                xt = xts[jb]
                i = 4 * jb + il
                qkv_ps = psmisc.tile([P, NQKV], F32, name=f"qkv{i}", tag="mi")
                for c in range(8):
                    nc.tensor.matmul(
                        qkv_ps,
                        lhsT=xt[:, c, ts(il, P)],
                        rhs=w_sb[:, c, :],
                        start=(c == 0),
                        stop=(c == 7),
                    )
                # stage q,k (f32) and v (bf16); square+reduce for RMS stats
                nc.vector.tensor_copy(qkv_sb[:, i, :], qkv_ps[:, 0 : 5 * HD])
                nc.vector.tensor_copy(v_sb[:, i, HD:P], qkv_ps[:, 5 * HD : NQKV])
                sq = wk.tile([P, 5 * HD], F32, name=f"sq{i}", tag="sq")
                nc.gpsimd.tensor_mul(sq, qkv_sb[:, i, :], qkv_sb[:, i, :])
                nc.vector.reduce_sum(
                    ss_all[:, 5 * i : 5 * i + 5],
                    sq.rearrange("p (h d) -> p h d", d=HD),
                    axis=AXX,
                )
            return go

        def u_post(jb):
            def go():
                # rsqrt via bitcast magic seed + 2 Newton steps (DVE)
                ssc = ss_all[:, 20 * jb : 20 * jb + 20]
                rc = r_all[:, 20 * jb : 20 * jb + 20]
                mm = wk.tile([P, 20], F32, name=f"m{jb}", tag="m")
                nc.vector.tensor_scalar(
                    out=mm, in0=ssc, scalar1=1.0 / HD, scalar2=RMS_EPS,
                    op0=ALU.mult, op1=ALU.add,
                )
                tt = wk.tile([P, 20], F32, name=f"t{jb}", tag="t")
                nc.vector.tensor_scalar(
                    out=tt.bitcast(I32), in0=mm.bitcast(I32),
                    scalar1=1, scalar2=-1,
                    op0=ALU.logical_shift_right, op1=ALU.bitwise_xor,
                )
                nc.vector.tensor_scalar(
                    out=rc.bitcast(I32), in0=tt.bitcast(I32),
                    scalar1=0x5F3759E0, scalar2=None, op0=ALU.add,
                )
                for _ in range(2):
                    nc.vector.tensor_mul(tt, rc, rc)
                    nc.vector.tensor_mul(tt, tt, mm)
                    nc.vector.tensor_scalar(
                        out=tt, in0=tt, scalar1=-0.5, scalar2=1.5,
                        op0=ALU.mult, op1=ALU.add,
                    )
                    nc.vector.tensor_mul(rc, rc, tt)
                rcv = rc.rearrange("p (t h) -> p t h", h=5)
                nc.vector.tensor_mul(
                    rcv[:, :, 0:NH], rcv[:, :, 0:NH],
                    qg8_sb[:, None, :].broadcast_to([P, 4, NH]),
                )
                # RoPE, mostly on the Pool engine (it is otherwise idle; the
                # DVE is loaded with masks/normalization/copies)
                qc = qkv_sb[:, 4 * jb : 4 * jb + 4, :]
                qcv = qc.rearrange("p t (h d) -> p (t h) d", d=HD)
                qks = rwk.tile([P, 4, 5 * HD], F32, name=f"qks{jb}", tag="qks")
                qksv = qks.rearrange("p t (h d) -> p (t h) d", d=HD)
                nc.vector.tensor_mul(
                    qksv, qcv,
                    rcv.rearrange("p t h -> p (t h)")[:, :, None].broadcast_to([P, 20, HD]),
                )
                rot = rwk.tile([P, 4, 5 * HD], BF16, name=f"rot{jb}", tag="rot")
                qks4 = qks.rearrange("p t (h d) -> p t h d", d=HD)
                rot4 = rot.rearrange("p t (h d) -> p t h d", d=HD)
                cosb = cos_sb[:, 4 * jb : 4 * jb + 4, None, :].broadcast_to(
                    [P, 4, 5, HD]
                )
                nc.vector.tensor_mul(rot4, qks4, cosb)
                rotv = rot.rearrange("p t (h d) -> p (t h) d", d=HD)
                sinb = sin_sb[:, 4 * jb : 4 * jb + 4, None, :].broadcast_to(
                    [P, 4, 5, 32]
                )
                m2a = rwk.tile([P, 20, 32], BF16, name=f"m2a{jb}", tag="m2a")
                nc.gpsimd.tensor_mul(
                    m2a.rearrange("p (t h) d -> p t h d", h=5),
                    qks4[:, :, :, 32:HD], sinb,
                )
                m2b = rwk.tile([P, 20, 32], BF16, name=f"m2b{jb}", tag="m2b")
                nc.gpsimd.tensor_mul(
                    m2b.rearrange("p (t h) d -> p t h d", h=5),
                    qks4[:, :, :, 0:32], sinb,
                )
                nc.vector.tensor_add(rotv[:, :, 0:32], rotv[:, :, 0:32], m2a)
                nc.vector.tensor_sub(rotv[:, :, 32:HD], rotv[:, :, 32:HD], m2b)
                rot_tiles[jb] = rot
            return go

        def tr_chunk(jb):
            rot = rot_tiles[jb]
            for il in range(4):
                i = 4 * jb + il
                for pair in range(2):
                    trp = psmisc.tile([P, P], BF16, name=f"tr{i}_{pair}", tag="mi")
                    nc.tensor.transpose(trp, rot[:, il, ts(pair, P)], ident)
                    nc.vector.tensor_copy(qT2[:, pair, ts(i, P)], trp)
                trk = psmisc.tile([HD, P], BF16, name=f"trk{i}", tag="mi")
                nc.tensor.transpose(trk, rot[:, il, 4 * HD : 5 * HD], ident)
                nc.vector.tensor_copy(kTe[0:HD, ts(i, P)], trk)
            nc.gpsimd.dma_start(
                out=kTo[HD:P, 4 * jb * P : (4 * jb + 4) * P],
                in_=kTe[0:HD, 4 * jb * P : (4 * jb + 4) * P],
            )

        def c_tile(j, mtile):
            def go():
                op = psmisc.tile([P, JW], F32, name=f"op{j}_{mtile}", tag="mi")
                for c in range(2):
                    nc.tensor.matmul(
                        op,
                        lhsT=wp_sb[:, c, ts(mtile, P)],
                        rhs=y_sb[:, c, ts(j, JW)],
                        start=(c == 0),
                        stop=(c == 1),
                    )
                o_sb = nwk.tile([P, JW], BF16, name=f"o{j}_{mtile}", tag="o")
                if mtile % 2 == 0:
                    nc.vector.tensor_copy(o_sb, op)
                else:
                    nc.scalar.copy(o_sb, op)
                [nc.sync, nc.scalar, nc.gpsimd][mtile % 3].dma_start(
                    out=ypt[ts(mtile, P), ts(j, JW)], in_=o_sb
                )
            return go

        def b_block(j, fillers):
            """Attention for q-block j; pops one filler after each tile."""
            nt = 4 * (j + 1)
            for pair in range(2):
                yp = psy.tile([P, 2, JW], F32, name=f"y{j}_{pair}", tag="y")
                pend = []

                def pv_flush():
                    pt, pw_, pc0 = pend.pop(0)
                    nc.tensor.matmul(
                        yp[:, 0, pc0:JW], lhsT=v_sb[:, pt, :], rhs=pw_[:, 0, pc0:JW],
                        start=(pt == 0), stop=(pt == nt - 1),
                    )
                    nc.tensor.matmul(
                        yp[:, 1, pc0:JW], lhsT=v_sb[:, pt, :], rhs=pw_[:, 1, pc0:JW],
                        start=(pt == 0), stop=(pt == nt - 1),
                    )

                for t in range(nt):
                    m = t - 4 * j
                    w = JW if m < 0 else JW - P * m
                    c0 = JW - w
                    st = psst.tile([P, 2, JW], F32, name=f"st{j}_{pair}_{t}", tag="st")
                    p_sb = pwk.tile([P, 2, JW], BF16, name=f"p{j}_{pair}_{t}", tag="p")
                    qe = qT2[:, pair, ts(j, JW)]
                    diag = m >= 0
                    nc.tensor.matmul(
                        st[:, 0, c0:JW], lhsT=kTe[:, ts(t, P)], rhs=qe[:, c0:JW],
                        start=True, stop=not diag, skip_group_check=diag,
                    )
                    nc.tensor.matmul(
                        st[:, 1, c0:JW], lhsT=kTo[:, ts(t, P)], rhs=qe[:, c0:JW],
                        start=True, stop=not diag, skip_group_check=diag,
                    )
                    if diag:
                        nc.tensor.matmul(
                            st[:, 0, c0 : c0 + P], lhsT=negI, rhs=tri_sb[:, 0:P],
                            start=False, stop=True, skip_group_check=True,
                        )
                        nc.tensor.matmul(
                            st[:, 1, c0 : c0 + P], lhsT=negI, rhs=tri_sb[:, 0:P],
                            start=False, stop=True, skip_group_check=True,
                        )
                    if fillers:
                        fillers.pop(0)()
                    if len(pend) >= 2:
                        pv_flush()
                    nc.scalar.activation(p_sb[:, :, c0:JW], st[:, :, c0:JW], ACT.Exp)
                    pend.append((t, p_sb, c0))
                while pend:
                    pv_flush()
                # softmax normalization: row 0 of each head-half holds the
                # denominator; y dims sit at rows 64-127 (aligned).
                slot = 2 * j + pair
                rcp = nwk.tile([1, 2, JW], F32, name=f"rc{j}_{pair}", tag="rcp")
                nc.vector.reciprocal_approx_fast(rcp, yp[0:1, :, :])
                nc.sync.dma_start(out=dnb[slot : slot + 1, :], in_=rcp)
                bc = nwk.tile([P, 2, JW], F32, name=f"bc{j}_{pair}", tag="bc")
                nc.gpsimd.dma_start(
                    out=bc[HD:P, :, :],
                    in_=dnb[slot : slot + 1, :].rearrange("o (h w) -> o h w", h=2)
                    .to_broadcast([HD, 2, JW]),
                )
                ytp = nwk.tile([P, 2, JW], BF16, name=f"yt{j}_{pair}", tag="yt")
                nc.vector.tensor_mul(
                    ytp[HD:P, :, :], yp[HD:P, :, :], bc[HD:P, :, :]
                )
                nc.sync.dma_start(
                    out=y_sb[0:HD, pair, ts(j, JW)], in_=ytp[HD:P, 0, :]
                )
                nc.sync.dma_start(
                    out=y_sb[HD:P, pair, ts(j, JW)], in_=ytp[HD:P, 1, :]
                )

        # program order (v7-style software pipeline): transposes decoupled
        # from their U chunks, C blocks pulled ahead of the final B block.
        for il in range(4):
            u_tile(0, il)()
        u_post(0)()
        tr_chunk(0)
        for il in range(4):
            u_tile(1, il)()
        b_block(0, [])
        u_post(1)()
        for il in range(4):
            u_tile(2, il)()
        tr_chunk(1)
        b_block(1, [])
        u_post(2)()
        for il in range(4):
            u_tile(3, il)()
        tr_chunk(2)
        for m in range(8):
            c_tile(0, m)()
        b_block(2, [u_post(3)])
        tr_chunk(3)
        for m in range(8):
            c_tile(1, m)()
        for m in range(8):
            c_tile(2, m)()
        b_block(3, [])
        for m in range(8):
            c_tile(3, m)()


_PROG = None


def _get_program():
    global _PROG
    if _PROG is None:
        _PROG = _build_program()
    return _PROG


def _bf16(a):
    import ml_dtypes

    return np.ascontiguousarray(a.astype(ml_dtypes.bfloat16))


def _host_tables():
    inv_freq = (
        1.0 / (ROPE_BASE ** (np.arange(0, HD, 2, dtype=np.float32) / HD))
    ).astype(np.float32)
    t = np.arange(S, dtype=np.float32)
    freqs = t[:, None] * inv_freq[None, :]  # [S, 32]
    cosf = np.cos(freqs).astype(np.float32)
    sinf = np.sin(freqs).astype(np.float32)
    cosd = np.concatenate([cosf, cosf], axis=1)  # [S, 64]
    cos1 = np.ascontiguousarray(
        cosd.reshape(NST, P, HD).transpose(1, 0, 2).reshape(P, NST * HD)
    )
    sin1 = np.ascontiguousarray(
        sinf.reshape(NST, P, 32).transpose(1, 0, 2).reshape(P, NST * 32)
    )
    p_idx = np.arange(P)[:, None]
    x_idx = np.arange(JW)[None, :]
    # complement causal mask: +30000 where masked; subtracted from scores via
    # a PE accumulate matmul with -I, so exp underflows to exactly 0 there.
    tri = _bf16(np.where(x_idx < p_idx, 30000.0, 0.0))  # [128, 512]
    return cos1, sin1, tri


def _in_maps(x, Wq, Wk, Wv, Wproj, q_gain):
    cos1, sin1, tri = _host_tables()
    maps = []
    for core in range(NC):
        b, g = divmod(core, KV)
        xT = _bf16(x[b].T)  # [D, S]
        wqkv = _bf16(
            np.concatenate(
                [
                    Wq[g * GD : (g + 1) * GD].T,
                    Wk[g * HD : (g + 1) * HD].T,
                    Wv[g * HD : (g + 1) * HD].T,
                ],
                axis=1,
            )
        )  # [D, 384]
        wsl = Wproj[:, g * GD : (g + 1) * GD].T.reshape(NH, HD, D)  # [head, d, m]
        wp2 = _bf16(
            np.stack(
                [
                    np.concatenate([wsl[0], wsl[1]], axis=0),
                    np.concatenate([wsl[2], wsl[3]], axis=0),
                ],
                axis=1,
            ).reshape(P, 2 * D)
        )
        qg8 = np.ascontiguousarray(
            (q_gain[g * NH : (g + 1) * NH] / 8.0).astype(np.float32).reshape(1, NH)
        )
        maps.append(
            {
                "xT": xT,
                "wqkv": wqkv,
                "wp2": wp2,
                "cos1": cos1,
                "sin1": sin1,
                "tri": tri,
                "qg8": qg8,
            }
        )
    return maps


def kernel(x, Wq, Wk, Wv, Wproj, q_gain, _collect=None):
    x = np.asarray(x, dtype=np.float32)
    Wq = np.asarray(Wq, dtype=np.float32)
    Wk = np.asarray(Wk, dtype=np.float32)
    Wv = np.asarray(Wv, dtype=np.float32)
    Wproj = np.asarray(Wproj, dtype=np.float32)
    q_gain = np.asarray(q_gain, dtype=np.float32)

    nc = _get_program()
    maps = _in_maps(x, Wq, Wk, Wv, Wproj, q_gain)
    res = run_bass_kernel_spmd(nc, maps, core_ids=list(range(NC)))
    if _collect is not None:
        _collect.append(res)

    out = np.zeros((B, S, D), dtype=np.float32)
    for core in range(NC):
        b, _ = divmod(core, KV)
        out[b] += res.results[core]["ypt"].astype(np.float32).T
    return out.astype(np.float32)

